# revision 5
# baseline (speedup 1.0000x reference)
"""Trainium2 Bass kernel for nn_AttentionLayer (additive attention layer).

Computes, for hidden (B,1,H), enc_seq (B,S,H), mask (B,S):
    pre    = enc_seq @ w0[:H] + hidden @ w0[H:] + b0      # (B,S,H)
    scores = tanh(pre) @ w1 (+ b1, dropped: softmax shift-invariant)
    attn   = softmax(where(mask, scores, -inf))           # (B,S)
    out    = einsum('bs,bsh->bh', attn, enc_seq)          # (B,H)

Sharding: data-parallel over batch across 8 NeuronCores (4 batches/core),
linear weights replicated. All matmuls run as fp32r (FP22 multiply, fp32
accumulate) on the PE at full rate.

Per-core plan:
  pass 1 (per 512-wide s-tile): PE-transpose enc 128x128 blocks -> encT
    (h_in on partitions), matmul with resident w0a -> preT in PSUM
    (h_out on partitions, s free), ScalarE tanh with per-partition bias
    v[h_out] = (hidden @ w0[H:] + b0) fused, then M=1 matmuls with w1
    columns accumulating scores (1, 512) in PSUM.
  softmax per batch on the (1, 2048) score row (mask applied via
  predicated copy); attn transposed back to columns via PE.
  pass 2: stream enc again in natural layout; out_row += attnT.T @ enc
    accumulated over s-chunks in PSUM; single row DMA per batch.
"""

import numpy as np

import concourse.bacc as bacc
import concourse.tile as tile
from concourse import mybir
from concourse.bass import ts
from concourse.bass_utils import run_bass_kernel_spmd
from concourse.masks import make_identity

F32 = mybir.dt.float32
F32R = mybir.dt.float32r
U8 = mybir.dt.uint8
AF = mybir.ActivationFunctionType
AX = mybir.AxisListType
ALU = mybir.AluOpType

N_CORES = 8
P = 128
B, S, H = 32, 2048, 1024
B_LOC = B // N_CORES          # 4 batches per core
KC = H // P                   # 8 contraction chunks
MC = H // P                   # 8 output-h chunks
ST = 512                      # s-tile (matmul free dim)
JT = ST // P                  # 4 128-blocks per s-tile
UT = S // ST                  # 4 s-tiles per batch
NU = B_LOC * UT               # 16 s-tile units per core
SC = S // P                   # 16 s-chunks per batch (pass 2)
NEG_BIG = -1.0e30


def _body(tc):
    nc = tc.nc
    enc = nc.dram_tensor("enc", [B_LOC, S, H], F32R, kind="ExternalInput").ap()
    hid = nc.dram_tensor("hid", [B_LOC, H], F32R, kind="ExternalInput").ap()
    msk = nc.dram_tensor("msk", [B_LOC, S], U8, kind="ExternalInput").ap()
    w0 = nc.dram_tensor("w0", [2 * H, H], F32R, kind="ExternalInput").ap()
    w1 = nc.dram_tensor("w1", [H], F32R, kind="ExternalInput").ap()
    b0 = nc.dram_tensor("b0", [H], F32, kind="ExternalInput").ap()
    out = nc.dram_tensor("out", [B_LOC, H], F32, kind="ExternalOutput").ap()

    # s = 512*u + 128*j + p  within a batch
    enc_r = enc.rearrange("b (u j p) h -> b u p j h", p=P, j=JT)
    w0a_r = w0[:H].rearrange("(o p) h -> p o h", p=P)
    w0b_r = w0[H:].rearrange("(o p) h -> p o h", p=P)

    with (
        tc.tile_pool(name="singles", bufs=1) as singles,
        tc.tile_pool(name="init", bufs=1) as init_pool,
        tc.tile_pool(name="w0bm", bufs=2) as w0bm_pool,
        tc.tile_pool(name="encload", bufs=4) as encload,
        tc.tile_pool(name="encT", bufs=2) as encT_pool,
        tc.tile_pool(name="tanh", bufs=1) as tanh_pool,
        tc.tile_pool(name="small", bufs=1) as small,
        tc.tile_pool(name="ps_tp", bufs=2, space="PSUM") as ps_tp,
        tc.tile_pool(name="ps_pre", bufs=2, space="PSUM") as ps_pre,
        tc.tile_pool(name="ps_sc", bufs=1, space="PSUM") as ps_sc,
        tc.tile_pool(name="ps_at", bufs=1, space="PSUM") as ps_at,
        tc.tile_pool(name="ps_nh", bufs=2, space="PSUM") as ps_nh,
    ):
        # ---- constants
        ident_f = singles.tile([P, P], F32)
        make_identity(nc, ident_f)
        ident = singles.tile([P, P], F32R)
        nc.vector.tensor_copy(ident[:], ident_f[:])

        w0a = singles.tile([P, KC, H], F32R)
        nc.sync.dma_start(out=w0a[:], in_=w0a_r)
        w1T = singles.tile([P, MC], F32R)
        nc.sync.dma_start(out=w1T[:], in_=w1.rearrange("(o p) -> p o", p=P))
        b0T = singles.tile([P, MC], F32)
        nc.sync.dma_start(out=b0T[:], in_=b0.rearrange("(o p) -> p o", p=P))

        # ---- v[h_out, b] = hidden[b] @ w0b + b0, kept as (h_out-part, b) cols
        hidn = init_pool.tile([B_LOC, H], F32R)
        nc.sync.dma_start(out=hidn[:], in_=hid[:])
        hid_ps = ps_tp.tile([P, KC * B_LOC], F32R, tag="tp")
        for k in range(KC):
            nc.tensor.transpose(
                hid_ps[:, k * B_LOC:(k + 1) * B_LOC],
                hidn[:, ts(k, P)],
                ident[:B_LOC, :B_LOC],
            )
        hiT = init_pool.tile([P, KC * B_LOC], F32R)
        nc.vector.tensor_copy(hiT[:], hid_ps[:])

        v_ps = ps_pre.tile([P, MC * B_LOC], F32, tag="pre")
        for m in range(MC):
            w0bm = w0bm_pool.tile([P, KC, P], F32R, tag="w0bm")
            nc.sync.dma_start(out=w0bm[:], in_=w0b_r[:, :, ts(m, P)])
            for k in range(KC):
                nc.tensor.matmul(
                    v_ps[:, m * B_LOC:(m + 1) * B_LOC],
                    w0bm[:, k, :],
                    hiT[:, k * B_LOC:(k + 1) * B_LOC],
                    start=(k == 0),
                    stop=(k == KC - 1),
                )
        v_sb = singles.tile([P, MC * B_LOC], F32)
        nc.vector.tensor_copy(v_sb[:], v_ps[:])
        for m in range(MC):
            nc.vector.tensor_tensor(
                v_sb[:, m * B_LOC:(m + 1) * B_LOC],
                v_sb[:, m * B_LOC:(m + 1) * B_LOC],
                b0T[:, m:m + 1].to_broadcast((P, B_LOC)),
                ALU.add,
            )

        # ---- pipelined unit helpers
        def load_enc(b, g):
            t = encload.tile([P, JT, H], F32R, tag="encload")
            nc.sync.dma_start(out=t[:], in_=enc_r[b, g])
            return t

        def emit_tp_group(enc1_t, encT_t, k):
            # transpose 4 (s=128, h=128) blocks of chunk k into encT[:, k, :]
            tp = ps_tp.tile([P, ST], F32R, tag="tp")
            for j in range(JT):
                nc.tensor.transpose(
                    tp[:, ts(j, P)], enc1_t[:, j, ts(k, P)], ident[:]
                )
            nc.vector.tensor_copy(encT_t[:, k, :], tp[:])

        def batch_tail(b, scores_sb):
            msk_sb = small.tile([1, S], U8, tag="msk")
            nc.sync.dma_start(out=msk_sb[:], in_=msk[b:b + 1, :])
            masked = small.tile([1, S], F32, tag="masked")
            nc.vector.memset(masked[:], NEG_BIG)
            nc.vector.copy_predicated(masked[:], msk_sb[:], scores_sb[:])
            mx = small.tile([1, 1], F32, tag="mx")
            nc.vector.reduce_max(out=mx[:], in_=masked[:], axis=AX.X)
            negmx = small.tile([1, 1], F32, tag="negmx")
            nc.vector.tensor_scalar_mul(negmx[:], mx[:], -1.0)
            sume = small.tile([1, 1], F32, tag="sume")
            nc.scalar.activation(
                out=masked[:], in_=masked[:], func=AF.Exp,
                bias=negmx[:], scale=1.0, accum_out=sume[:],
            )
            rinv = small.tile([1, 1], F32, tag="rinv")
            nc.vector.reciprocal(rinv[:], sume[:])
            # attn lives on row 0 of a 4-row tile; rows 1-3 are never read
            # back (fp32r transpose needs the 4x128 -> 128x4 shape).
            attn = small.tile([4, S], F32R, tag="attn")
            nc.vector.tensor_scalar_mul(attn[0:1, :], masked[:], rinv[:])

            at_ps = ps_at.tile([P, 4 * SC], F32R, tag="at")
            for j in range(SC):
                nc.tensor.transpose(
                    at_ps[:, 4 * j:4 * j + 4], attn[0:4, ts(j, P)], ident[:4, :4]
                )
            attnT = small.tile([P, SC], F32R, tag="attnT")
            nc.vector.tensor_copy(
                attnT[:], at_ps.rearrange("p (j f) -> p j f", f=4)[:, :, 0]
            )

            # pass 2: out[b] = attn @ enc[b]
            nh_ps = [
                ps_nh.tile([1, 512], F32, tag="nh", name=f"nh_{n}")
                for n in range(2)
            ]
            for g in range(UT):
                enc2_t = load_enc(b, g)
                for j in range(JT):
                    sj = g * JT + j
                    for n in range(2):
                        nc.tensor.matmul(
                            nh_ps[n][:],
                            attnT[:, sj:sj + 1],
                            enc2_t[:, j, ts(n, 512)],
                            start=(sj == 0),
                            stop=(sj == SC - 1),
                        )
            nh_sb = small.tile([1, H], F32, tag="nh_sb")
            for n in range(2):
                nc.vector.tensor_copy(nh_sb[0:1, ts(n, 512)], nh_ps[n][:])
            nc.sync.dma_start(out=out[b:b + 1, :], in_=nh_sb[:])

        # ---- main loop over s-tile units, software-pipelined
        enc1_tiles = {
            0: load_enc(0, 0),
            1: load_enc(0, 1),
        }
        encT_cur = encT_pool.tile([P, KC, ST], F32R, tag="encT")
        for k in range(KC):
            emit_tp_group(enc1_tiles[0], encT_cur, k)

        scores_sb = None
        for u in range(NU):
            b, st = divmod(u, UT)
            if st == 0:
                scores_sb = small.tile([1, S], F32, tag="scores")
            if u + 2 < NU:
                b2, st2 = divmod(u + 2, UT)
                enc1_tiles[u + 2] = load_enc(b2, st2)
            encT_nxt = None
            if u + 1 < NU:
                encT_nxt = encT_pool.tile([P, KC, ST], F32R, tag="encT")

            tanh_t = tanh_pool.tile([P, MC, ST], F32R, tag="tanh")
            for m in range(MC):
                if encT_nxt is not None:
                    emit_tp_group(enc1_tiles[u + 1], encT_nxt, m)
                pre_ps = ps_pre.tile([P, ST], F32, tag="pre")
                for k in range(KC):
                    nc.tensor.matmul(
                        pre_ps[:],
                        w0a[:, k, ts(m, P)],
                        encT_cur[:, k, :],
                        start=(k == 0),
                        stop=(k == KC - 1),
                    )
                nc.scalar.activation(
                    out=tanh_t[:, m, :], in_=pre_ps[:], func=AF.Tanh,
                    bias=v_sb[:, m * B_LOC + b:m * B_LOC + b + 1], scale=1.0,
                )
            sc_ps = ps_sc.tile([1, ST], F32, tag="sc")
            for m in range(MC):
                nc.tensor.matmul(
                    sc_ps[:],
                    w1T[:, m:m + 1],
                    tanh_t[:, m, :],
                    start=(m == 0),
                    stop=(m == MC - 1),
                )
            nc.vector.tensor_copy(scores_sb[0:1, ts(st, ST)], sc_ps[:])

            enc1_tiles.pop(u, None)
            encT_cur = encT_nxt
            if st == UT - 1:
                batch_tail(b, scores_sb)


_NC_CACHE = None


def _build_nc():
    global _NC_CACHE
    if _NC_CACHE is None:
        nc = bacc.Bacc("TRN2", target_bir_lowering=False, debug=False)
        with tile.TileContext(nc) as tc:
            _body(tc)
        nc.compile()
        _NC_CACHE = nc
    return _NC_CACHE


def _make_in_maps(hidden, enc_seq, mask, w0, b0, w1):
    hidden = np.ascontiguousarray(np.asarray(hidden, dtype=np.float32)).reshape(B, H)
    enc_seq = np.ascontiguousarray(np.asarray(enc_seq, dtype=np.float32))
    mask_u8 = np.ascontiguousarray(np.asarray(mask).astype(np.uint8))
    w0 = np.ascontiguousarray(np.asarray(w0, dtype=np.float32))
    b0 = np.ascontiguousarray(np.asarray(b0, dtype=np.float32)).reshape(H)
    w1 = np.ascontiguousarray(np.asarray(w1, dtype=np.float32)).reshape(H)
    in_maps = []
    for c in range(N_CORES):
        sl = slice(c * B_LOC, (c + 1) * B_LOC)
        in_maps.append({
            "enc": enc_seq[sl],
            "hid": hidden[sl],
            "msk": mask_u8[sl],
            "w0": w0,
            "w1": w1,
            "b0": b0,
        })
    return in_maps


def kernel(hidden, enc_seq, mask, w0, b0, w1, b1):
    nc = _build_nc()
    in_maps = _make_in_maps(hidden, enc_seq, mask, w0, b0, w1)
    res = run_bass_kernel_spmd(nc, in_maps, core_ids=list(range(N_CORES)))
    outs = [res.results[c]["out"] for c in range(N_CORES)]
    return np.concatenate(outs, axis=0).astype(np.float32)
